# revision 3
# baseline (speedup 1.0000x reference)
"""DenseNGCNLayer Trainium2 kernel: out = A^3 (X @ W) + bias.

8-core SPMD sharding: destination nodes row-partitioned across cores.
Each spmm hop: dma_gather of source feature rows (bf16, source-quadrant
passes so gather indices fit int16) + PE matmuls with on-chip-built
one-hot "indicator" matrices performing the segment-sum into PSUM,
accumulated across passes in SBUF. Halo exchange = HBM AllGather of the
per-core row slices between hops. The dense weight is applied once up
front (replicated W).
"""
import numpy as np
import ml_dtypes

import concourse.bacc as bacc
import concourse.mybir as mybir
from concourse.tile import TileContext
from concourse.bass_utils import run_bass_kernel_spmd

BF16 = mybir.dt.bfloat16
F32 = mybir.dt.float32

N_CORES = 8
NHOPS = 3
D = 128
SLOT = 64

# defaults for the real problem; configure() recomputes deriveds
N = 100000
DIN = 512
NQUAD = 4
GROUP = 12


def configure(n=100000, din=512, nquad=4, group=12):
    global N, DIN, NQUAD, GROUP
    global SLICE, SLICE_PAD, NPAIRS, QROWS, NSLOTS, NGROUPS
    N, DIN, NQUAD, GROUP = n, din, nquad, group
    SLICE = N // N_CORES
    SLICE_PAD = -(-SLICE // 128) * 128
    NPAIRS = SLICE_PAD // 128
    QROWS = N // NQUAD
    assert QROWS <= 32767
    NSLOTS = SLICE_PAD // SLOT
    NGROUPS = -(-NSLOTS // GROUP)


configure()

_cache = {}


def _chunk_base_of(C):
    chunk_base = np.zeros((NQUAD, NSLOTS + 1), np.int64)
    for q in range(NQUAD):
        prev = chunk_base[q - 1, NSLOTS] if q > 0 else 0
        chunk_base[q, 0] = prev
        chunk_base[q, 1:] = prev + np.cumsum(C[q])
    return chunk_base


def _build_program(C, do_finalize=True):
    """C: int array [NQUAD, NSLOTS] chunk capacities (shared across cores)."""
    chunk_base = _chunk_base_of(C)
    TOT = int(chunk_base[NQUAD - 1, NSLOTS])
    KC = DIN // 128

    nc = bacc.Bacc("TRN2", num_devices=N_CORES, num_swdge_queues=4)
    t_xt = nc.dram_tensor("xt", [DIN, SLICE_PAD], BF16, kind="ExternalInput")
    t_w = nc.dram_tensor("w", [DIN, D], BF16, kind="ExternalInput")
    t_bias = nc.dram_tensor("biasr", [128, D], F32, kind="ExternalInput")
    t_iota = nc.dram_tensor("iota", [128, SLOT], BF16, kind="ExternalInput")
    t_idx = nc.dram_tensor("idxw", [128, TOT * 8], mybir.dt.int16, kind="ExternalInput")
    t_drel = nc.dram_tensor("drel", [128, TOT], BF16, kind="ExternalInput")
    t_val = nc.dram_tensor("vals", [128, TOT], BF16, kind="ExternalInput")
    t_out = nc.dram_tensor("out_slice", [SLICE_PAD, D], F32, kind="ExternalOutput")
    t_slice = [nc.dram_tensor(f"slice{h}", [SLICE_PAD, D], BF16, kind="Internal")
               for h in range(NHOPS)]
    t_tab = [nc.dram_tensor(f"tab{h}", [N, D], BF16, kind="Internal")
             for h in range(NHOPS)]

    with TileContext(nc) as tc:
        with (
            tc.tile_pool(name="const", bufs=1) as constp,
            tc.tile_pool(name="acc", bufs=1) as accp,
        ):
            iota = constp.tile([128, SLOT], BF16)
            nc.sync.dma_start(iota[:], t_iota[:])
            drel = constp.tile([128, TOT], BF16)
            nc.sync.dma_start(drel[:], t_drel[:])
            vals = constp.tile([128, TOT], BF16)
            nc.sync.dma_start(vals[:], t_val[:])
            biasr = constp.tile([128, D], F32)
            nc.sync.dma_start(biasr[:], t_bias[:])

            accs = []
            for i in range(NPAIRS):
                acc_t = accp.tile([128, D], F32, tag=f"acc{i}")
                accs.append(acc_t)

            # ---------------- phase 1: slice of X @ W ----------------
            with (
                tc.tile_pool(name="xtp", bufs=1) as xtp,
                tc.tile_pool(name="wp", bufs=1) as wp,
                tc.tile_pool(name="p1ev", bufs=4) as p1ev,
                tc.tile_pool(name="p1ps", bufs=8, space="PSUM") as p1ps,
            ):
                wt = wp.tile([128, KC, D], BF16)
                nc.sync.dma_start(
                    wt[:], t_w.ap().rearrange("(c p) n -> p c n", p=128))
                xts = []
                for kc in range(KC):
                    xt_t = xtp.tile([128, SLICE_PAD], BF16, tag=f"xt{kc}")
                    nc.sync.dma_start(xt_t[:], t_xt[kc * 128:(kc + 1) * 128, :])
                    xts.append(xt_t)
                for i in range(NPAIRS):
                    ps = p1ps.tile([128, D], F32, tag="p1ps")
                    for kc in range(KC):
                        nc.tensor.matmul(
                            ps[:], xts[kc][:, i * 128:(i + 1) * 128],
                            wt[:, kc, :], start=(kc == 0), stop=(kc == KC - 1))
                    ev = p1ev.tile([128, D], BF16, tag="p1ev")
                    nc.vector.tensor_copy(ev[:], ps[:])
                    nc.sync.dma_start(t_slice[0][i * 128:(i + 1) * 128, :], ev[:])
            nc.gpsimd.collective_compute(
                kind="AllGather", op=mybir.AluOpType.bypass,
                replica_groups=[list(range(N_CORES))],
                ins=[t_slice[0].ap()[0:SLICE]], outs=[t_tab[0].ap()])

            # ---------------- hops ----------------
            with (
                tc.tile_pool(name="idxp", bufs=3) as idxp,
                tc.tile_pool(name="gat", bufs=3) as gatp,
                tc.tile_pool(name="ind", bufs=3) as indp,
                tc.tile_pool(name="ev", bufs=4) as evp,
                tc.tile_pool(name="ps", bufs=8, space="PSUM") as psp,
            ):
                call_i = 0
                for h in range(NHOPS):
                    tab = t_tab[h]
                    for g in range(NGROUPS):
                        s0 = g * GROUP
                        s1 = min(s0 + GROUP, NSLOTS)
                        for q in range(NQUAD):
                            c0 = int(chunk_base[q, s0])
                            ct = int(chunk_base[q, s1] - c0)
                            nidx = ct * 128
                            it = idxp.tile([128, nidx // 16], mybir.dt.int16,
                                           tag="it")
                            nc.sync.dma_start(
                                it[:], t_idx[:, c0 * 8: c0 * 8 + nidx // 16])
                            gt = gatp.tile([128, ct, D], BF16, tag="gt")
                            nc.gpsimd.dma_gather(
                                gt[:], tab.ap()[q * QROWS:(q + 1) * QROWS],
                                it[:], nidx, nc.gpsimd.to_reg(nidx), D,
                                single_packet=False, queue_num=call_i % 4)
                            call_i += 1
                            ind = indp.tile([128, ct, SLOT], BF16, tag="ind")
                            nc.vector.tensor_tensor(
                                ind[:],
                                drel[:, c0:c0 + ct][:, :, None]
                                .broadcast_to([128, ct, SLOT]),
                                iota[:, None, :].broadcast_to([128, ct, SLOT]),
                                mybir.AluOpType.is_equal)
                            nc.vector.tensor_tensor(
                                ind[:], ind[:],
                                vals[:, c0:c0 + ct][:, :, None]
                                .broadcast_to([128, ct, SLOT]),
                                mybir.AluOpType.mult)
                            for s in range(s0, s1):
                                st = int(chunk_base[q, s]) - c0
                                Cqs = int(C[q, s])
                                ps = psp.tile([SLOT, D], F32, tag="ps")
                                for k in range(Cqs):
                                    nc.tensor.matmul(
                                        ps[:], ind[:, st + k, :],
                                        gt[:, st + k, :],
                                        start=(k == 0), stop=(k == Cqs - 1))
                                acc_ap = accs[s // 2][(s % 2) * SLOT:
                                                      (s % 2 + 1) * SLOT, :]
                                if q == 0:
                                    nc.vector.tensor_copy(acc_ap, ps[:])
                                else:
                                    nc.vector.tensor_add(acc_ap, acc_ap, ps[:])
                    # eviction
                    if h < NHOPS - 1:
                        for i in range(NPAIRS):
                            ev = evp.tile([128, D], BF16, tag="evb")
                            nc.vector.tensor_copy(ev[:], accs[i][:])
                            nc.sync.dma_start(
                                t_slice[h + 1][i * 128:(i + 1) * 128, :], ev[:])
                        nc.gpsimd.collective_compute(
                            kind="AllGather", op=mybir.AluOpType.bypass,
                            replica_groups=[list(range(N_CORES))],
                            ins=[t_slice[h + 1].ap()[0:SLICE]],
                            outs=[t_tab[h + 1].ap()])
                    else:
                        for i in range(NPAIRS):
                            ev = evp.tile([128, D], F32, tag="evf")
                            nc.vector.tensor_add(ev[:], accs[i][:], biasr[:])
                            nc.sync.dma_start(
                                t_out[i * 128:(i + 1) * 128, :], ev[:])
    if do_finalize:
        nc.finalize()
    else:
        nc.compile()
    return nc


def _prep_inputs(adj_indices, adj_values, features, weight, bias):
    row = np.asarray(adj_indices[0], dtype=np.int64)
    col = np.asarray(adj_indices[1], dtype=np.int64)
    val = np.asarray(adj_values, dtype=np.float32)
    E = row.shape[0]

    m = row // SLICE
    q = col // QROWS
    d_loc = row - m * SLICE
    slot = d_loc // SLOT
    key = (m * NQUAD + q) * NSLOTS + slot
    order = np.argsort(key, kind="stable")
    skey = key[order]
    ngroups_total = N_CORES * NQUAD * NSLOTS
    counts = np.bincount(key, minlength=ngroups_total)
    starts = np.concatenate([[0], np.cumsum(counts)])
    rank = np.arange(E) - starts[skey]

    cnt = counts.reshape(N_CORES, NQUAD, NSLOTS)
    C = np.maximum(1, -(-cnt // 128)).max(axis=0)  # [NQUAD, NSLOTS]
    chunk_base = _chunk_base_of(C)
    TOT = int(chunk_base[NQUAD - 1, NSLOTS])

    sq = q[order]
    ss = slot[order]
    sm = m[order]
    pos = (chunk_base[sq, ss] * 128 + rank).astype(np.int64)
    sidx = (col[order] - sq * QROWS).astype(np.int16)
    sdrel = (d_loc[order] - ss * SLOT).astype(np.float32)
    sval = val[order]

    call_slices = []
    for g in range(NGROUPS):
        s0, s1 = g * GROUP, min((g + 1) * GROUP, NSLOTS)
        for qq in range(NQUAD):
            call_slices.append((int(chunk_base[qq, s0]), int(chunk_base[qq, s1])))

    bf = ml_dtypes.bfloat16
    in_maps = []
    x_t = np.ascontiguousarray(np.asarray(features, dtype=np.float32).T)
    w_bf = np.asarray(weight, dtype=np.float32).astype(bf)
    bias_rep = np.tile(np.asarray(bias, dtype=np.float32).reshape(1, D), (128, 1))
    iota_np = np.tile(np.arange(SLOT, dtype=np.float32).astype(bf), (128, 1))

    for c in range(N_CORES):
        sel = sm == c
        idxf = np.zeros(TOT * 128, np.int16)
        drelf = np.zeros(TOT * 128, np.float32)
        valf = np.zeros(TOT * 128, np.float32)
        p = pos[sel]
        idxf[p] = sidx[sel]
        drelf[p] = sdrel[sel]
        valf[p] = sval[sel]
        wr = np.empty((16, TOT * 8), np.int16)
        for (a, b) in call_slices:
            nn = (b - a) * 128
            wr[:, a * 8: a * 8 + nn // 16] = idxf[a * 128: b * 128].reshape(
                nn // 16, 16).T
        idx_wrapped = np.tile(wr, (8, 1))
        xt = np.zeros((DIN, SLICE_PAD), np.float32)
        xt[:, :SLICE] = x_t[:, c * SLICE:(c + 1) * SLICE]
        in_maps.append({
            "xt": xt.astype(bf),
            "w": w_bf,
            "biasr": bias_rep,
            "iota": iota_np,
            "idxw": idx_wrapped,
            "drel": drelf.reshape(TOT, 128).T.astype(bf).copy(),
            "vals": valf.reshape(TOT, 128).T.astype(bf).copy(),
        })
    return C, in_maps


def kernel(adj_indices, adj_values, features, weight, bias):
    C, in_maps = _prep_inputs(adj_indices, adj_values, features, weight, bias)
    ckey = C.tobytes()
    if ckey not in _cache:
        _cache[ckey] = _build_program(C)
    nc = _cache[ckey]
    res = run_bass_kernel_spmd(nc, in_maps, core_ids=list(range(N_CORES)))
    if res.exec_time_ns is not None:
        print(f"HW exec time: {res.exec_time_ns} ns")
    out = np.concatenate(
        [res.results[c]["out_slice"][:SLICE] for c in range(N_CORES)], axis=0)
    return out.astype(np.float32)


# revision 5
# speedup vs baseline: 1.1506x; 1.1506x over previous
"""DenseNGCNLayer Trainium2 kernel: out = A^3 (X @ W) + bias.

8-core SPMD sharding: destination nodes row-partitioned across cores.
Each spmm hop: dma_gather of source feature rows (bf16, source-quadrant
passes so gather indices fit int16) + PE matmuls with on-chip-built
one-hot "indicator" matrices performing the segment-sum into PSUM,
accumulated across passes in SBUF. Halo exchange = HBM AllGather of the
per-core row slices between hops. The dense weight is applied once up
front (replicated W).

Edge groups are padded to 32-index granularity (capacity shared across
cores so the SPMD program is static); PE matmuls use partition-offset
segments so groups need not align to 128-partition chunk boundaries.
"""
import numpy as np
import ml_dtypes

import concourse.bacc as bacc
import concourse.mybir as mybir
from concourse.tile import TileContext
from concourse.bass_utils import run_bass_kernel_spmd

BF16 = mybir.dt.bfloat16
F32 = mybir.dt.float32

N_CORES = 8
NHOPS = 3
D = 128
SLOT = 64

N = 100000
DIN = 512
NQUAD = 4
GROUP = 12
GRAN = 64          # edge-group padding granularity (partition bases must be 0/32/64)


def configure(n=100000, din=512, nquad=4, group=12):
    global N, DIN, NQUAD, GROUP
    global SLICE, SLICE_PAD, NPAIRS, QROWS, NSLOTS, NGROUPS
    N, DIN, NQUAD, GROUP = n, din, nquad, group
    SLICE = N // N_CORES
    SLICE_PAD = -(-SLICE // 128) * 128
    NPAIRS = SLICE_PAD // 128
    QROWS = N // NQUAD
    assert QROWS <= 32767
    NSLOTS = SLICE_PAD // SLOT
    NGROUPS = -(-NSLOTS // GROUP)


configure()

_cache = {}


def _layout_of(L):
    """L: [NQUAD, NSLOTS] padded edge counts (multiples of GRAN).

    Returns (call_list, TOT): call_list[i] = (q, g, c0, ct, offs) with
    c0 = global 128-chunk base of the call, ct = chunks in the call,
    offs = [(s, off, n), ...] edge offset/length of slot s within call.
    """
    calls = []
    cb = 0
    for g in range(NGROUPS):
        s0, s1 = g * GROUP, min((g + 1) * GROUP, NSLOTS)
        for q in range(NQUAD):
            offs = []
            off = 0
            for s in range(s0, s1):
                offs.append((s, off, int(L[q, s])))
                off += int(L[q, s])
            ct = -(-off // 128)
            calls.append((q, g, cb, ct, offs))
            cb += ct
    return calls, cb


def _build_program(L, do_finalize=True, repeat=1):
    calls, TOT = _layout_of(L)
    KC = DIN // 128

    nc = bacc.Bacc("TRN2", num_devices=N_CORES, num_swdge_queues=4)
    t_xt = nc.dram_tensor("xt", [DIN, SLICE_PAD], BF16, kind="ExternalInput")
    t_w = nc.dram_tensor("w", [DIN, D], BF16, kind="ExternalInput")
    t_bias = nc.dram_tensor("biasr", [128, D], F32, kind="ExternalInput")
    t_iota = nc.dram_tensor("iota", [128, SLOT], BF16, kind="ExternalInput")
    t_idx = nc.dram_tensor("idxw", [128, TOT * 8], mybir.dt.int16, kind="ExternalInput")
    t_drel = nc.dram_tensor("drel", [128, TOT], BF16, kind="ExternalInput")
    t_val = nc.dram_tensor("vals", [128, TOT], BF16, kind="ExternalInput")
    t_out = nc.dram_tensor("out_slice", [SLICE_PAD, D], F32, kind="ExternalOutput")
    t_slice = [nc.dram_tensor(f"slice{h}", [SLICE_PAD, D], BF16, kind="Internal")
               for h in range(NHOPS)]
    t_tab = [nc.dram_tensor(f"tab{h}", [N, D], BF16, kind="Internal")
             for h in range(NHOPS)]

    with TileContext(nc) as tc:
        with (
            tc.tile_pool(name="const", bufs=1) as constp,
            tc.tile_pool(name="acc", bufs=1) as accp,
        ):
            iota = constp.tile([128, SLOT], BF16)
            nc.sync.dma_start(iota[:], t_iota[:])
            drel = constp.tile([128, TOT], BF16)
            nc.sync.dma_start(drel[:], t_drel[:])
            vals = constp.tile([128, TOT], BF16)
            nc.sync.dma_start(vals[:], t_val[:])
            biasr = constp.tile([128, D], F32)
            nc.sync.dma_start(biasr[:], t_bias[:])

            accs = []
            for i in range(NPAIRS):
                acc_t = accp.tile([128, D], F32, tag=f"acc{i}")
                accs.append(acc_t)

            # ---------------- phase 1: slice of X @ W ----------------
            with (
                tc.tile_pool(name="xtp", bufs=1) as xtp,
                tc.tile_pool(name="wp", bufs=1) as wp,
                tc.tile_pool(name="p1ev", bufs=4) as p1ev,
                tc.tile_pool(name="p1ps", bufs=8, space="PSUM") as p1ps,
            ):
                wt = wp.tile([128, KC, D], BF16)
                nc.sync.dma_start(
                    wt[:], t_w.ap().rearrange("(c p) n -> p c n", p=128))
                xts = []
                for kc in range(KC):
                    xt_t = xtp.tile([128, SLICE_PAD], BF16, tag=f"xt{kc}")
                    nc.sync.dma_start(xt_t[:], t_xt[kc * 128:(kc + 1) * 128, :])
                    xts.append(xt_t)
                for i in range(NPAIRS):
                    ps = p1ps.tile([128, D], F32, tag="p1ps")
                    for kc in range(KC):
                        nc.tensor.matmul(
                            ps[:], xts[kc][:, i * 128:(i + 1) * 128],
                            wt[:, kc, :], start=(kc == 0), stop=(kc == KC - 1))
                    ev = p1ev.tile([128, D], BF16, tag="p1ev")
                    nc.vector.tensor_copy(ev[:], ps[:])
                    nc.sync.dma_start(t_slice[0][i * 128:(i + 1) * 128, :], ev[:])
            nc.gpsimd.collective_compute(
                kind="AllGather", op=mybir.AluOpType.bypass,
                replica_groups=[list(range(N_CORES))],
                ins=[t_slice[0].ap()[0:SLICE]], outs=[t_tab[0].ap()])

            # ---------------- hops ----------------
            with (
                tc.tile_pool(name="idxp", bufs=3) as idxp,
                tc.tile_pool(name="gat", bufs=3) as gatp,
                tc.tile_pool(name="ind", bufs=3) as indp,
                tc.tile_pool(name="ev", bufs=4) as evp,
                tc.tile_pool(name="ps", bufs=8, space="PSUM") as psp,
            ):
                call_i = 0
                for rep in range(repeat):
                    for h in range(NHOPS):
                        tab = t_tab[h]
                        for (q, g, c0, ct, offs) in calls:
                            nidx = ct * 128
                            it = idxp.tile([128, nidx // 16], mybir.dt.int16,
                                           tag="it")
                            nc.sync.dma_start(
                                it[:], t_idx[:, c0 * 8: c0 * 8 + nidx // 16])
                            gt = gatp.tile([128, ct, D], BF16, tag="gt")
                            nc.gpsimd.dma_gather(
                                gt[:], tab.ap()[q * QROWS:(q + 1) * QROWS],
                                it[:], nidx, nc.gpsimd.to_reg(nidx), D,
                                single_packet=False, queue_num=call_i % 4)
                            call_i += 1
                            ind = indp.tile([128, ct, SLOT], BF16, tag="ind")
                            nc.vector.tensor_tensor(
                                ind[:],
                                drel[:, c0:c0 + ct][:, :, None]
                                .broadcast_to([128, ct, SLOT]),
                                iota[:, None, :].broadcast_to([128, ct, SLOT]),
                                mybir.AluOpType.is_equal)
                            nc.vector.tensor_tensor(
                                ind[:], ind[:],
                                vals[:, c0:c0 + ct][:, :, None]
                                .broadcast_to([128, ct, SLOT]),
                                mybir.AluOpType.mult)
                            for (s, off, n) in offs:
                                ps = psp.tile([SLOT, D], F32, tag="ps")
                                segs = []
                                p = off
                                while p < off + n:
                                    colx = p // 128
                                    p0 = p % 128
                                    take = min(128 - p0, off + n - p)
                                    segs.append((colx, p0, take))
                                    p += take
                                for si, (colx, p0, take) in enumerate(segs):
                                    nc.tensor.matmul(
                                        ps[:],
                                        ind[p0:p0 + take, colx, :],
                                        gt[p0:p0 + take, colx, :],
                                        start=(si == 0),
                                        stop=(si == len(segs) - 1))
                                acc_ap = accs[s // 2][(s % 2) * SLOT:
                                                      (s % 2 + 1) * SLOT, :]
                                if q == 0:
                                    nc.vector.tensor_copy(acc_ap, ps[:])
                                else:
                                    nc.vector.tensor_add(acc_ap, acc_ap, ps[:])
                        # eviction
                        if h < NHOPS - 1:
                            for i in range(NPAIRS):
                                ev = evp.tile([128, D], BF16, tag="evb")
                                nc.vector.tensor_copy(ev[:], accs[i][:])
                                nc.sync.dma_start(
                                    t_slice[h + 1][i * 128:(i + 1) * 128, :],
                                    ev[:])
                            nc.gpsimd.collective_compute(
                                kind="AllGather", op=mybir.AluOpType.bypass,
                                replica_groups=[list(range(N_CORES))],
                                ins=[t_slice[h + 1].ap()[0:SLICE]],
                                outs=[t_tab[h + 1].ap()])
                        else:
                            for i in range(NPAIRS):
                                ev = evp.tile([128, D], F32, tag="evf")
                                nc.vector.tensor_add(ev[:], accs[i][:], biasr[:])
                                nc.sync.dma_start(
                                    t_out[i * 128:(i + 1) * 128, :], ev[:])
    if do_finalize:
        nc.finalize()
    else:
        nc.compile()
    return nc


def _prep_inputs(adj_indices, adj_values, features, weight, bias):
    row = np.asarray(adj_indices[0], dtype=np.int64)
    col = np.asarray(adj_indices[1], dtype=np.int64)
    val = np.asarray(adj_values, dtype=np.float32)
    E = row.shape[0]

    m = row // SLICE
    q = col // QROWS
    d_loc = row - m * SLICE
    slot = d_loc // SLOT
    key = (m * NQUAD + q) * NSLOTS + slot
    order = np.argsort(key, kind="stable")
    skey = key[order]
    ngroups_total = N_CORES * NQUAD * NSLOTS
    counts = np.bincount(key, minlength=ngroups_total)
    starts = np.concatenate([[0], np.cumsum(counts)])
    rank = np.arange(E) - starts[skey]

    cnt = counts.reshape(N_CORES, NQUAD, NSLOTS)
    L = GRAN * np.maximum(1, -(-cnt // GRAN)).max(axis=0)  # [NQUAD, NSLOTS]
    calls, TOT = _layout_of(L)

    base_qs = np.zeros((NQUAD, NSLOTS), np.int64)
    for (qq, g, c0, ct, offs) in calls:
        for (s, off, n) in offs:
            base_qs[qq, s] = c0 * 128 + off

    sq = q[order]
    ss = slot[order]
    sm = m[order]
    pos = base_qs[sq, ss] + rank
    sidx = (col[order] - sq * QROWS).astype(np.int16)
    sdrel = (d_loc[order] - ss * SLOT).astype(np.float32)
    sval = val[order]

    bf = ml_dtypes.bfloat16
    in_maps = []
    x_t = np.ascontiguousarray(np.asarray(features, dtype=np.float32).T)
    w_bf = np.asarray(weight, dtype=np.float32).astype(bf)
    bias_rep = np.tile(np.asarray(bias, dtype=np.float32).reshape(1, D), (128, 1))
    iota_np = np.tile(np.arange(SLOT, dtype=np.float32).astype(bf), (128, 1))

    for c in range(N_CORES):
        sel = sm == c
        idxf = np.zeros(TOT * 128, np.int16)
        drelf = np.zeros(TOT * 128, np.float32)
        valf = np.zeros(TOT * 128, np.float32)
        p = pos[sel]
        idxf[p] = sidx[sel]
        drelf[p] = sdrel[sel]
        valf[p] = sval[sel]
        wr = np.empty((16, TOT * 8), np.int16)
        for (qq, g, a, ctc, offs) in calls:
            nn = ctc * 128
            wr[:, a * 8: a * 8 + nn // 16] = idxf[a * 128: (a + ctc) * 128].reshape(
                nn // 16, 16).T
        idx_wrapped = np.tile(wr, (8, 1))
        xt = np.zeros((DIN, SLICE_PAD), np.float32)
        xt[:, :SLICE] = x_t[:, c * SLICE:(c + 1) * SLICE]
        in_maps.append({
            "xt": xt.astype(bf),
            "w": w_bf,
            "biasr": bias_rep,
            "iota": iota_np,
            "idxw": idx_wrapped,
            "drel": drelf.reshape(TOT, 128).T.astype(bf).copy(),
            "vals": valf.reshape(TOT, 128).T.astype(bf).copy(),
        })
    return L, in_maps


def kernel(adj_indices, adj_values, features, weight, bias):
    L, in_maps = _prep_inputs(adj_indices, adj_values, features, weight, bias)
    ckey = L.tobytes()
    if ckey not in _cache:
        _cache[ckey] = _build_program(L)
    nc = _cache[ckey]
    res = run_bass_kernel_spmd(nc, in_maps, core_ids=list(range(N_CORES)))
    if res.exec_time_ns is not None:
        print(f"HW exec time: {res.exec_time_ns} ns")
    out = np.concatenate(
        [res.results[c]["out_slice"][:SLICE] for c in range(N_CORES)], axis=0)
    return out.astype(np.float32)
